# revision 12
# baseline (speedup 1.0000x reference)
"""Trainium2 Bass kernel for LocalXLAttention (chunk-summed variant), v2.

Math: the reference einsum sums over the chunk index z, so every query
attends to the same three [w, dh] K/V matrices built from chunk sums:
  K_prev = sum(chunks 0..6), K_cur = sum(all 8), K_next = sum(chunks 1..7)
(identically for V).  Per sequence position l and head h:
  attn[l,h,:]  = qp[l,h,:] @ KbigT          (KbigT: [dh, 3w])
  probs        = softmax(attn, axis=-1)
  ctx[l,h,:]   = probs[l,h,:] @ Vbig        (Vbig:  [3w, dh])
  out          = ctx.reshape(L, dm) @ Wc

Sharding: L=4096 split 512 rows per core over 8 cores; each core's L-shard
is exactly one attention chunk, so each core projects only its own kv
chunk ([dh x 512] k and v) and a single small AllReduce (with prev/next
masking folded into the contribution) builds [K_prev|K_cur|K_next] and the
V sums on every core.  No replicated 16MB kv stream.

Inputs are fed as bf16 (PE runs bf16 at 1 cycle/row, same as fp32r, and
DMA bytes halve).  Accumulations are fp32 in PSUM.  Probs normalization is
deferred to the context via an all-ones column of Vbig; the per-(l,h)
reciprocal denominator is broadcast across partitions with a rank-1 PE
matmul (no DRAM round-trip).  Output projection runs in two head-halves
so the first half overlaps the attention loop.
"""

import sys
for _p in ('/opt/pypackages', '/opt/trn_rl_repo'):
    if _p not in sys.path:
        sys.path.insert(0, _p)

import numpy as np
import ml_dtypes

import concourse.bass as bass
import concourse.bacc as bacc
import concourse.tile as tile
from concourse import mybir
from concourse.bass_utils import run_bass_kernel_spmd
from concourse.masks import make_identity

F32 = mybir.dt.float32
F32R = mybir.dt.float32r
BF16 = mybir.dt.bfloat16
AF = mybir.ActivationFunctionType

N_CORES = 8
L = 4096          # full sequence
LS = L // N_CORES # 512 rows per core == one attention chunk
DM = 1024
NH = 16
DH = 64
W = 512           # chunk width
J3 = 3 * W        # 1536 softmax width
NJ = J3 // 128    # 12 j-chunks
DMT = DM // 128   # 8 dm-chunks


def build_nc():
    nc = bacc.Bacc(None, target_bir_lowering=False, num_devices=N_CORES)

    qT = nc.dram_tensor("qT", [DM, LS], BF16, kind="ExternalInput")
    kvT = nc.dram_tensor("kvT", [DM, LS], BF16, kind="ExternalInput")
    Wq = nc.dram_tensor("Wq", [DM, DM], BF16, kind="ExternalInput")
    Wkv = nc.dram_tensor("Wkv", [DM, 2 * DH], BF16, kind="ExternalInput")
    Wc = nc.dram_tensor("Wc", [DM, DM], BF16, kind="ExternalInput")
    cmask = nc.dram_tensor("cmask", [128, 2], F32, kind="ExternalInput")
    out = nc.dram_tensor("out", [LS, DM], BF16, kind="ExternalOutput")

    with tile.TileContext(nc) as tc:
        with tc.tile_pool(name="weights", bufs=8) as wpool, \
             tc.tile_pool(name="small", bufs=1) as spool, \
             tc.tile_pool(name="qp", bufs=8) as qpool, \
             tc.tile_pool(name="qpt", bufs=4) as qptpool, \
             tc.tile_pool(name="probs", bufs=4) as ppool, \
             tc.tile_pool(name="ctxu", bufs=8) as cpool, \
             tc.tile_pool(name="part", bufs=8) as partpool, \
             tc.tile_pool(name="norm", bufs=4) as npool, \
             tc.tile_pool(name="misc", bufs=2) as mpool, \
             tc.tile_pool(name="dram", bufs=2, space="DRAM") as dpool, \
             tc.tile_pool(name="psmm", bufs=2, space="PSUM") as psmm, \
             tc.tile_pool(name="psacc", bufs=2, space="PSUM") as psacc, \
             tc.tile_pool(name="pssm", bufs=2, space="PSUM") as pssm:

            # ---------- warm the exp table off the critical path ----------
            warm = spool.tile([1, 8], F32, tag="warm")
            nc.vector.memset(warm, 0.0)
            nc.scalar.activation(warm, warm, AF.Exp)

            # ---------- DMAs (kv + wkv first: they gate the collective) ----
            kvt_sb = []
            for d in range(DMT):
                t = wpool.tile([128, LS], BF16, tag="kvt", name=f"kvt{d}")
                nc.sync.dma_start(out=t, in_=kvT[128 * d:128 * (d + 1), :])
                kvt_sb.append(t)
            wkv_sb = []
            for d in range(DMT):
                t = wpool.tile([128, 2 * DH], BF16, tag="wkv", name=f"wkv{d}")
                nc.scalar.dma_start(out=t, in_=Wkv[128 * d:128 * (d + 1), :])
                wkv_sb.append(t)
            cm_sb = spool.tile([128, 2], F32, tag="cmask")
            nc.sync.dma_start(out=cm_sb, in_=cmask[:, :])

            wq_sb = []
            for d in range(DMT):
                t = wpool.tile([128, DM], BF16, tag="wq", name=f"wq{d}")
                nc.gpsimd.dma_start(out=t, in_=Wq[128 * d:128 * (d + 1), :])
                wq_sb.append(t)
            qt_sb = []
            for d in range(DMT):
                t = qpool.tile([128, LS], BF16, tag="qt", name=f"qt{d}")
                nc.scalar.dma_start(out=t, in_=qT[128 * d:128 * (d + 1), :])
                qt_sb.append(t)
            wc_sb = []
            for d in range(DMT):
                t = wpool.tile([128, DM], BF16, tag="wc", name=f"wc{d}")
                nc.gpsimd.dma_start(out=t, in_=Wc[128 * d:128 * (d + 1), :])
                wc_sb.append(t)

            ident = spool.tile([128, 128], F32, tag="ident")
            make_identity(nc, ident)

            # ---------- own-chunk K,V projection: kc = Wkv.T @ kvT_shard ----
            # kc_ps[p, y]: rows 0:64 = k, rows 64:128 = v for this core's chunk
            kc_ps = psacc.tile([128, W], F32, tag="acc", name="kc")
            for d in range(DMT):
                nc.tensor.matmul(kc_ps, wkv_sb[d], kvt_sb[d],
                                 start=(d == 0), stop=(d == DMT - 1))

            # ---------- collective input: [prev-masked | cur | next-masked] --
            cc_in = spool.tile([128, J3], F32, tag="ccin")
            nc.vector.tensor_scalar_mul(cc_in[:, 0:W], kc_ps, cm_sb[:, 0:1])
            nc.vector.tensor_copy(cc_in[:, W:2 * W], kc_ps)
            nc.vector.tensor_scalar_mul(cc_in[:, 2 * W:3 * W], kc_ps, cm_sb[:, 1:2])

            ccin_d = dpool.tile([128, J3], F32, name="ccin_d", tag="ccin", bufs=1)
            ccout_d = dpool.tile([128, J3], F32, name="ccout_d", tag="ccout", bufs=1)
            nc.sync.dma_start(out=ccin_d, in_=cc_in)
            nc.gpsimd.collective_compute(
                "AllReduce",
                mybir.AluOpType.add,
                replica_groups=[list(range(N_CORES))],
                ins=[ccin_d[:].opt()],
                outs=[ccout_d[:].opt()],
            )
            ccb = spool.tile([128, J3], F32, tag="ccb")
            nc.sync.dma_start(out=ccb, in_=ccout_d)

            # ---------- KbigT [64, 1536] duplicated into partitions 64:128 ----
            kbig = spool.tile([128, J3], F32R, tag="kbig")
            nc.vector.tensor_copy(kbig[0:DH, :], ccb[0:DH, :])
            nc.vector.tensor_copy(kbig[DH:2 * DH, :], ccb[0:DH, :])

            # ---------- Vbig [128, 12, 65(+pad)]: col 64 = ones ----------
            vbig = spool.tile([128, NJ, 68], F32R, tag="vbig")
            ones_sb = spool.tile([128, DH], F32, tag="ones")
            nc.vector.memset(ones_sb, 1.0)
            for j in range(NJ):
                nc.vector.tensor_copy(vbig[:, j, DH:DH + 1], ones_sb[:, 0:1])
            # V rows of ccb sit at partition base 64; transpose lhsT must share
            # the identity's base-0 partitions, so stage them at base 0 first.
            vtmp = spool.tile([DH, J3], F32, tag="vtmp")
            nc.vector.tensor_copy(vtmp, ccb[DH:2 * DH, :])
            for j in range(NJ):
                tp = pssm.tile([128, W], F32, tag="tp")
                nc.tensor.transpose(tp[:, 0:DH],
                                    vtmp[:, 128 * j:128 * (j + 1)],
                                    ident[0:DH, 0:DH])
                nc.vector.tensor_copy(vbig[:, j, 0:DH], tp[:, 0:DH])

            # ---------- QP_T = Wq.T @ q.T (unscaled; 1/8 folded into exp) ----
            qpt_sb = []
            for t4 in range(4):
                ps = psmm.tile([128, 1024], F32, tag="mm")
                for half in range(2):
                    hd = 2 * t4 + half
                    for d in range(DMT):
                        nc.tensor.matmul(
                            ps[:, 512 * half:512 * (half + 1)],
                            wq_sb[d][:, 128 * hd:128 * (hd + 1)],
                            qt_sb[d],
                            start=(d == 0), stop=(d == DMT - 1))
                sb = qptpool.tile([128, 1024], F32R, tag="qpt")
                nc.vector.tensor_copy(sb, ps)
                qpt_sb.append(sb)

            # ---------- attention: QK -> exp -> PV, pair-packed ----------
            ctxu_sb = []  # 8 pair tiles [128, 512] bf16: rows 0:64 head 2t, 64:128 head 2t+1
            part_sb = []  # first-half out-proj partials [128, 512] f32
            # per-(l,h) reciprocal denominators bounce through DRAM so one DMA
            # can broadcast them across partitions (stride-0 partition read).
            rscr = dpool.tile([8, 2 * W], F32, name="rscr", tag="rscr", bufs=1)

            def out_partial(he_lo, he_he, fuse_add):
                """accumulate out-proj over he in [he_lo, he_he) for all
                (lt, half); either stash partials or fuse with stashed."""
                for lt in range(LS // 128):
                    for half in range(2):
                        ps = pssm.tile([128, W], F32, tag="tp")
                        for k, he in enumerate(range(he_lo, he_he)):
                            nc.tensor.matmul(
                                ps,
                                ctxu_sb[he][:, 128 * lt:128 * (lt + 1)],
                                wc_sb[he][:, 512 * half:512 * (half + 1)],
                                start=(k == 0), stop=(he == he_he - 1))
                        if not fuse_add:
                            pt = partpool.tile([128, W], F32, tag="part")
                            nc.vector.tensor_copy(pt, ps)
                            part_sb.append(pt)
                        else:
                            ob = mpool.tile([128, W], BF16, tag="ob", bufs=2)
                            nc.vector.tensor_add(ob, part_sb[2 * lt + half], ps)
                            nc.sync.dma_start(
                                out=out[128 * lt:128 * (lt + 1),
                                        512 * half:512 * (half + 1)],
                                in_=ob)

            for t in range(8):  # head pairs (2t, 2t+1)
                qpt = qpt_sb[t // 2]
                csl = slice(512 * (t % 2), 512 * (t % 2) + W)
                rhsA = qpt[0:DH, csl]
                rhsB = qpt[DH:2 * DH, csl]
                ctxA = psacc.tile([128, W], F32, tag="acc", name=f"ctxA{t}")
                ctxB = psacc.tile([128, W], F32, tag="acc", name=f"ctxB{t}")
                for j in range(NJ):
                    qk = psmm.tile([128, 1024], F32, tag="mm", name=f"qk{t}_{j}")
                    nc.tensor.matmul(qk[:, 0:W],
                                     kbig[0:DH, 128 * j:128 * (j + 1)],
                                     rhsA, start=True, stop=True)
                    nc.tensor.matmul(qk[:, W:2 * W],
                                     kbig[DH:2 * DH, 128 * j:128 * (j + 1)],
                                     rhsB, start=True, stop=True)
                    pr = ppool.tile([128, 1024], F32R, tag="probs", name=f"pr{t}_{j}")
                    nc.scalar.activation(pr, qk, AF.Exp, scale=0.125)
                    nc.tensor.matmul(ctxA[0:DH + 1, :], vbig[:, j, 0:DH + 1],
                                     pr[:, 0:W],
                                     start=(j == 0), stop=(j == NJ - 1))
                    nc.tensor.matmul(ctxB[0:DH + 1, :], vbig[:, j, 0:DH + 1],
                                     pr[:, W:2 * W],
                                     start=(j == 0), stop=(j == NJ - 1))

                # -------- normalize: 1/Z broadcast via DRAM round-trip ------
                # both heads' Z rows packed side-by-side at partition 0
                zz = npool.tile([1, 2 * W], F32, tag="zz", name=f"zz{t}")
                rz = npool.tile([1, 2 * W], F32, tag="rz", name=f"rz{t}")
                nc.vector.tensor_copy(zz[0:1, 0:W], ctxA[DH:DH + 1, :])
                nc.vector.tensor_copy(zz[0:1, W:2 * W], ctxB[DH:DH + 1, :])
                nc.vector.reciprocal_approx_fast(rz, zz)
                nc.scalar.dma_start(out=rscr[t:t + 1, :], in_=rz)
                ctxu = cpool.tile([128, W], BF16, tag="ctxu", name=f"ctxu{t}")
                nc.vector.tensor_copy(ctxu[0:DH, :], ctxA[0:DH, :])
                nc.vector.tensor_copy(ctxu[DH:2 * DH, :], ctxB[0:DH, :])
                bc = npool.tile([128, W], F32, tag="bc", name=f"bc{t}")
                src = bass.AP(tensor=rscr.tensor,
                              offset=rscr.offset + t * 2 * W,
                              ap=[[W, 2], [0, DH], [1, W]])
                nc.scalar.dma_start(out=bc, in_=src)
                nc.vector.tensor_mul(ctxu, ctxu, bc)
                ctxu_sb.append(ctxu)

                if t == 3:
                    out_partial(0, 4, fuse_add=False)
            out_partial(4, 8, fuse_add=True)

    nc.compile()
    return nc


_NC = None


def _get_nc():
    global _NC
    if _NC is None:
        _NC = build_nc()
    return _NC


def make_in_maps(q, kv, Wq, Wkv, Wc):
    q = np.asarray(q, dtype=np.float32)
    kv = np.asarray(kv, dtype=np.float32)
    bf = ml_dtypes.bfloat16
    qT_full = np.ascontiguousarray(q[0].T.astype(bf))     # [DM, L]
    kvT_full = np.ascontiguousarray(kv[0].T.astype(bf))   # [DM, L]
    Wq = np.ascontiguousarray(np.asarray(Wq, dtype=np.float32).astype(bf))
    Wkv = np.ascontiguousarray(np.asarray(Wkv, dtype=np.float32).astype(bf))
    Wc = np.ascontiguousarray(np.asarray(Wc, dtype=np.float32).astype(bf))

    in_maps = []
    for i in range(N_CORES):
        cm = np.empty((128, 2), dtype=np.float32)
        cm[:, 0] = 0.0 if i == N_CORES - 1 else 1.0   # contributes to prev-sum
        cm[:, 1] = 0.0 if i == 0 else 1.0             # contributes to next-sum
        in_maps.append({
            "qT": np.ascontiguousarray(qT_full[:, LS * i:LS * (i + 1)]),
            "kvT": np.ascontiguousarray(kvT_full[:, LS * i:LS * (i + 1)]),
            "Wq": Wq,
            "Wkv": Wkv,
            "Wc": Wc,
            "cmask": cm,
        })
    return in_maps


def kernel(q, kv, Wq, Wkv, Wc, w):
    assert int(w) == W
    q = np.asarray(q, dtype=np.float32)
    B = q.shape[0]
    assert B == 1 and q.shape[1] == L and q.shape[2] == DM

    in_maps = make_in_maps(q, kv, Wq, Wkv, Wc)
    nc = _get_nc()
    res = run_bass_kernel_spmd(nc, in_maps, list(range(N_CORES)))
    out = np.concatenate(
        [np.asarray(res.results[i]["out"], dtype=np.float32)
         for i in range(N_CORES)], axis=0)
    return out.reshape(1, L, DM)
